# revision 9
# baseline (speedup 1.0000x reference)
"""Trainium2 Bass kernel: per-sample dynamic conv (KernelAggregation).

Problem: out[b] = conv2d(x[b], sum_n att[b,n]*W[n], pad=1) + (att @ bias)[b]
  x: (16, 256, 56, 56) f32, att: (16, 8), W: (8, 256, 256, 3, 3), bias: (8, 256)

Sharding: data-parallel over batch, 2 samples per core across 8 cores.

Per-core device kernel:
  1. Stream the (host pre-transposed) weight bank once from DRAM; mix both
     samples' dynamic conv weights on VectorE via scalar_tensor_tensor FMA
     (w_mix[s] += att[s,n] * W[n]), in matmul-ready [ci, (ky,kx,co)] layout.
  2. Conv as 9 shifted matmuls over a 58-stride zero-padded input image:
     out[co, p] += w_mix[ci, kp, co].T @ xpad[ci, p + dy*58+dx], accumulated
     in PSUM over 2 ci-chunks x 9 taps; N-tiles of 464 px (8 rows).
  3. ScalarE adds the mixed bias (Identity activation, per-partition bias)
     while copying PSUM -> SBUF; DMA result rows (dropping the 2 pad cols).

Matmul dtype is float32r (TF32-like, full PE rate at N>=256) by default;
set _MM_DTYPE = "float32" for exact-fp32 (4x slower PE).
"""

import numpy as np
from contextlib import ExitStack

B, DIM, H, W = 16, 256, 56, 56
NK, KS = 8, 3
NCORES = 8
SPC = B // NCORES          # samples per core
S = W + 2                  # padded row stride (58)
NPAD = S * S               # 3364
XP_LEN = NPAD + 4          # slack so shifted reads stay in-bounds
ROWS_PER_T = 8
NT = H // ROWS_PER_T       # 7 spatial tiles
NTILE = ROWS_PER_T * S     # 464 (= matmul moving dim, <=512 fp32)
CI_CH = DIM // 128         # 2
CO_CH = DIM // 128         # 2
KK = KS * KS               # 9

_MM_DTYPE = "float32r"     # "float32r" | "float32" | "bfloat16"


def _imports():
    try:
        import concourse.bass as bass  # noqa: F401
    except ImportError:
        import sys
        for p in ("/opt/trn_rl_repo",):
            if p not in sys.path:
                sys.path.insert(0, p)
    import concourse.bass as bass
    import concourse.tile as tile
    from concourse import mybir
    from concourse.bass_utils import run_bass_kernel_spmd
    return bass, tile, mybir, run_bass_kernel_spmd


NBANK = 3   # bank streaming buffers
NTMP = 4    # ACT->DVE scaled-weight staging buffers
NPS = 4     # PSUM tiles
NOUT = 4    # output staging buffers


def build_bass_raw(mm_dtype_name=None):
    bass, tile, mybir, _ = _imports()
    dt = mybir.dt
    mm_dtype = getattr(dt, mm_dtype_name or _MM_DTYPE)
    nc = bass.Bass()

    x = nc.dram_tensor("x", [SPC, DIM, H, W], mm_dtype, kind="ExternalInput")
    wbank = nc.dram_tensor("wbank", [NK, CI_CH, 128, KK * DIM], dt.float32,
                           kind="ExternalInput")
    attb = nc.dram_tensor("attb", [128, SPC * NK], dt.float32,
                          kind="ExternalInput")
    bmixT = nc.dram_tensor("bmixT", [128, CO_CH * SPC], dt.float32,
                           kind="ExternalInput")
    y = nc.dram_tensor("y", [SPC, DIM, H, W], dt.float32, kind="ExternalOutput")

    ctx = ExitStack()
    with ctx:
        sb = lambda shape, name: ctx.enter_context(
            nc.sbuf_tensor(name, shape, dt.float32))
        sbm = lambda shape, name: ctx.enter_context(
            nc.sbuf_tensor(name, shape, mm_dtype))
        att_sb = sb([128, SPC * NK], "att_sb")
        bmix_sb = sb([128, CO_CH * SPC], "bmix_sb")
        xp = [[sbm([128, XP_LEN], f"xp{s}_{c}") for c in range(CI_CH)]
              for s in range(SPC)]
        wmix = [[sbm([128, KK * DIM], f"wm{s}_{c}") for c in range(CI_CH)]
                for s in range(SPC)]
        bank = [sb([128, KK * DIM], f"bank{i}") for i in range(NBANK)]
        tmp = [sb([128, KK * DIM], f"tmp{i}") for i in range(NTMP)]
        ot = [sb([128, NTILE], f"ot{i}") for i in range(NOUT)]
        psum = [ctx.enter_context(nc.psum_tensor(f"ps{i}", [128, NTILE],
                                                 dt.float32))
                for i in range(NPS)]

        sem = lambda name: ctx.enter_context(nc.semaphore(name))
        sem_small = sem("sem_small")   # att/bmix DMA done (2x16)
        sem_ms = sem("sem_ms")         # DVE memsets done (1 each, 4)
        sem_x = sem("sem_x")           # x interior DMA done (4x16)
        sem_bank = sem("sem_bank")     # bank DMA k done at 16*(k+1)
        sem_scale = sem("sem_scale")   # ACT weight scale-muls (1 each, 32)
        sem_mixop = sem("sem_mixop")   # DVE wmix copy/adds (1 each, 32)
        sem_mm = sem("sem_mm")         # PE per-out-tile group done (1 ea, 28)
        sem_act = sem("sem_act")       # ACT out bias-copies (1 each, 28)
        sem_outdma = sem("sem_outdma")  # out DMA done (16 each, 28)

        Copy = mybir.ActivationFunctionType.Copy
        Ident = mybir.ActivationFunctionType.Identity

        # ---------------- DVE: memsets, then wmix accumulate
        for i, (s, c) in enumerate([(s, c) for s in range(SPC)
                                    for c in range(CI_CH)]):
            ms_ap = xp[s][c][:]
            if mm_dtype != dt.float32:
                ms_ap = ms_ap.bitcast(dt.float32)  # memset lacks f32r ISA
            nc.vector.memset(ms_ap, 0.0).then_inc(sem_ms, 1)
        j = 0
        for k in range(NK * CI_CH):
            n, c = divmod(k, CI_CH)
            for s in range(SPC):
                nc.vector.wait_ge(sem_scale, j + 1)
                t = tmp[j % NTMP][:]
                if n == 0:
                    nc.vector.tensor_copy(wmix[s][c][:], t).then_inc(
                        sem_mixop, 1)
                else:
                    nc.vector.tensor_add(wmix[s][c][:], wmix[s][c][:],
                                         t).then_inc(sem_mixop, 1)
                j += 1

        # ---------------- GPSIMD: all input DMAs
        nc.gpsimd.dma_start(att_sb[:], attb[:, :]).then_inc(sem_small, 16)
        nc.gpsimd.dma_start(bmix_sb[:], bmixT[:, :]).then_inc(sem_small, 16)
        for k in range(min(NBANK, NK * CI_CH)):
            n, c = divmod(k, CI_CH)
            nc.gpsimd.dma_start(bank[k % NBANK][:],
                                wbank[n, c, :, :]).then_inc(sem_bank, 16)
        for i, (s, c) in enumerate([(s, c) for s in range(SPC)
                                    for c in range(CI_CH)]):
            nc.gpsimd.wait_ge(sem_ms, i + 1)
            interior = xp[s][c][:, :NPAD].rearrange(
                "p (r u) -> p r u", u=S)[:, 1:1 + H, 1:1 + W]
            nc.gpsimd.dma_start(
                interior, x[s, c * 128:(c + 1) * 128, :, :]).then_inc(sem_x, 16)
        for k in range(NBANK, NK * CI_CH):
            n, c = divmod(k, CI_CH)
            nc.gpsimd.wait_ge(sem_scale, 2 * (k - NBANK) + 2)
            nc.gpsimd.dma_start(bank[k % NBANK][:],
                                wbank[n, c, :, :]).then_inc(sem_bank, 16)

        # ---------------- ACT: weight scale-muls, then out bias-copies
        nc.scalar.wait_ge(sem_small, 32)
        j = 0
        for k in range(NK * CI_CH):
            n, c = divmod(k, CI_CH)
            nc.scalar.wait_ge(sem_bank, 16 * (k + 1))
            for s in range(SPC):
                if j >= NTMP:
                    nc.scalar.wait_ge(sem_mixop, j - NTMP + 1)
                nc.scalar.activation(
                    tmp[j % NTMP][:], bank[k % NBANK][:],
                    Copy, scale=att_sb[:, s * NK + n: s * NK + n + 1],
                ).then_inc(sem_scale, 1)
                j += 1
        tiles = [(s, t, co) for s in range(SPC) for t in range(NT)
                 for co in range(CO_CH)]
        for ti, (s, t, co) in enumerate(tiles):
            nc.scalar.wait_ge(sem_mm, ti + 1)
            if ti >= NOUT:
                nc.scalar.wait_ge(sem_outdma, 16 * (ti - NOUT + 1))
            nc.scalar.activation(
                ot[ti % NOUT][:], psum[ti % NPS][:], Ident,
                bias=bmix_sb[:, co * SPC + s: co * SPC + s + 1],
            ).then_inc(sem_act, 1)

        # ---------------- PE: conv matmuls
        nc.tensor.wait_ge(sem_x, 16 * SPC * CI_CH)
        nc.tensor.wait_ge(sem_mixop, SPC * NK * CI_CH)
        for ti, (s, t, co) in enumerate(tiles):
            if ti >= NPS:
                nc.tensor.wait_ge(sem_act, ti - NPS + 1)
            for c in range(CI_CH):
                for kp in range(KK):
                    off = (kp // 3) * S + (kp % 3) + t * NTILE
                    lhsT = wmix[s][c][:, kp * DIM + co * 128:
                                      kp * DIM + co * 128 + 128]
                    rhs = xp[s][c][:, off: off + NTILE]
                    mm = nc.tensor.matmul(
                        psum[ti % NPS][:], lhsT, rhs,
                        start=(c == 0 and kp == 0),
                        stop=(c == CI_CH - 1 and kp == KK - 1))
            mm.then_inc(sem_mm, 1)

        # ---------------- SYNC: output DMAs
        for ti, (s, t, co) in enumerate(tiles):
            nc.sync.wait_ge(sem_act, ti + 1)
            src = ot[ti % NOUT][:].rearrange("p (r u) -> p r u", u=S)[:, :, 0:W]
            nc.sync.dma_start(
                y[s, co * 128:(co + 1) * 128,
                  t * ROWS_PER_T:(t + 1) * ROWS_PER_T, :], src,
            ).then_inc(sem_outdma, 16)
        nc.sync.wait_ge(sem_outdma, 16 * len(tiles))
    return nc




def prep_inputs(x, attention, weight, bias):
    """Host-side sharding + layout prep. Returns per-core input maps."""
    x = np.ascontiguousarray(np.asarray(x, dtype=np.float32))
    attention = np.asarray(attention, dtype=np.float32)
    weight = np.asarray(weight, dtype=np.float32)
    bias = np.asarray(bias, dtype=np.float32)

    # (n, co, ci, ky, kx) -> (n, ci, ky, kx, co) -> [n, ci_ch, 128, kk*co]
    wb = np.ascontiguousarray(weight.transpose(0, 2, 3, 4, 1)).reshape(
        NK, CI_CH, 128, KK * DIM)
    # att broadcast across partitions: [128, B*NK]
    attb_all = np.ascontiguousarray(
        np.repeat(attention.reshape(1, B * NK), 128, axis=0))
    # host-mixed bias: bm = att @ bias; bmixT[p, co*SPC+s] = bm[s0+s, co*128+p]
    bm = attention @ bias

    in_maps = []
    for cidx in range(NCORES):
        s0 = cidx * SPC
        in_maps.append({
            "x": np.ascontiguousarray(x[s0:s0 + SPC]),
            "wbank": wb,
            "attb": np.ascontiguousarray(
                attb_all[:, s0 * NK:(s0 + SPC) * NK]),
            "bmixT": np.ascontiguousarray(
                bm[s0:s0 + SPC].reshape(SPC, CO_CH, 128).transpose(
                    2, 1, 0)).reshape(128, CO_CH * SPC),
        })
    return in_maps




def run(x, attention, weight, bias, trace=False, mm_dtype_name=None, **kw):
    _, _, _, run_bass_kernel_spmd = _imports()
    nc = build_bass_raw(mm_dtype_name)
    in_maps = prep_inputs(x, attention, weight, bias)
    res = run_bass_kernel_spmd(nc, in_maps, list(range(NCORES)),
                               trace=trace, **kw)
    y = np.concatenate([res.results[i]["y"] for i in range(NCORES)], axis=0)
    return y.astype(np.float32), res


def kernel(x, attention, weight, bias):
    y, _ = run(x, attention, weight, bias)
    return y
